# revision 10
# baseline (speedup 1.0000x reference)
"""KAN layer (B=8192, IN_F=OUT_F=1024, GRID=5) on 8 Trainium2 cores.

Math: Y[b,o] = W0[o]*silu(x) + spline_o(clip(x,-1,1)) + b[o], x = X[b,o].
The degree-1 B-spline is evaluated in the *segment* basis
    spline(clip(x)) = A''[o] + sum_j gamma_j[o] * v_j(x),
    v_j(x) = clip(x, s_{j-1}, s_j),  knots s = (-1,-0.5,0,0.5,1),
    gamma_j = w1 * m_j (segment slopes),
so each map is a 2-op tensor_scalar clip straight from x.

Sharding: edges across the 8 cores (128 edges/core, full batch 8192 on the
free dim).  Per core TensorE does a per-edge diagonal combine of 5 feature
maps into PSUM per 512-column chunk: two fp8e4 DoubleRow matmuls carry the
four spline maps (2 maps/pass, diag pairs prebuilt on host), one fp16
matmul carries silu.  ScalarE: silu + most of the PSUM evacuation
(Identity+bias); VectorE: fp8 clips (2x_1P, one SBUF port) + the evac
remainder; GpSimd: the v1 clips (and v4 on odd superblocks) — safe to
overlap since VectorE never uses 2-port modes.  I/O fp16; fp8 weight
quantization is minimax-compensated into the per-edge bias on host.
"""
import sys

for _p in ("/root/.axon_site", "/root/.axon_site/_ro/trn_rl_repo", "/root/.axon_site/_ro/pypackages"):
    if _p not in sys.path:
        sys.path.append(_p)

import numpy as np
import ml_dtypes

import concourse.bacc as bacc
import concourse.tile as tile
from concourse import mybir
from concourse.bass_utils import run_bass_kernel_spmd

B, IN_F, OUT_F, GRID = 8192, 1024, 1024, 5
N_CORES = 8
PER = OUT_F // N_CORES          # 128 edges per core
NB = B                          # 8192 batch columns per core
SBLK = 2048                     # superblock columns
NSB = NB // SBLK                # 4 superblocks
CHUNK = 512                     # one PSUM bank of fp32
SPLIT = 1568                    # evac columns on ScalarE per superblock
SPLIT_H = 784                   # evac columns on ScalarE per half-superblock

_nc_cache = None


def _build():
    f32 = mybir.dt.float32
    f16 = mybir.dt.float16
    f8 = mybir.dt.float8e4
    AF = mybir.ActivationFunctionType
    OP = mybir.AluOpType
    DRm = mybir.MatmulPerfMode.DoubleRow

    nc = bacc.Bacc("TRN2", target_bir_lowering=False, debug=False)
    xt = nc.dram_tensor("xt", [PER, NB], f16, kind="ExternalInput").ap()
    cpack = nc.dram_tensor("cpack", [PER, 8], f32, kind="ExternalInput").ap()
    dstat16 = nc.dram_tensor("dstat16", [PER, 128], f16, kind="ExternalInput").ap()
    dstat8 = nc.dram_tensor("dstat8", [PER, 512], f8, kind="ExternalInput").ap()
    yt = nc.dram_tensor("yt", [PER, NB], f16, kind="ExternalOutput").ap()

    with tile.TileContext(nc) as tc:
        with tc.tile_pool(name="const", bufs=1) as cpool, \
             tc.tile_pool(name="xin", bufs=1) as xpool, \
             tc.tile_pool(name="sil", bufs=1) as spool, \
             tc.tile_pool(name="pA", bufs=2) as apool, \
             tc.tile_pool(name="pB", bufs=2) as bpool, \
             tc.tile_pool(name="yout", bufs=2) as ypool, \
             tc.tile_pool(name="ps", bufs=2, space="PSUM") as pspool:
            # consts via the scalar-HWDGE ring; dp (first matmul dep) first
            dp = cpool.tile([128, 4, 128], f8)
            nc.scalar.dma_start(dp[:].rearrange("p a b -> p (a b)"), dstat8[:, :])
            d16 = cpool.tile([128, 128], f16)
            nc.scalar.dma_start(d16[:], dstat16[:, :])
            cp = cpool.tile([128, 8], f32)
            nc.scalar.dma_start(cp[:], cpack[:, :])
            dpA = dp[:, 0:2, :]
            dpB = dp[:, 2:4, :]
            dsil = d16[:]

            # input loads on the sync/HWDGE ring; small first chunk
            x0 = xpool.tile([128, SBLK], f16, tag="x0", name="x0")
            nc.sync.dma_start(x0[:, 0:1024], xt[:, 0:1024])
            nc.sync.dma_start(x0[:, 1024:SBLK], xt[:, 1024:SBLK])
            x1 = xpool.tile([128, SBLK], f16, tag="x1", name="x1")
            nc.sync.dma_start(x1[:], xt[:, SBLK:2 * SBLK])
            x23 = xpool.tile([128, 2 * SBLK], f16, tag="x23", name="x23")
            nc.sync.dma_start(x23[:], xt[:, 2 * SBLK:4 * SBLK])

            scr = cpool.tile([128, CHUNK], f16)
            nc.vector.memset(scr[:], 0.25)
            # trigger ACT table sets during the DMA ramp
            dum = cpool.tile([128, 2], f16)
            nc.scalar.activation(dum[:, 0:1], scr[:, 0:1], AF.Silu)
            nc.scalar.activation(dum[:, 1:2], scr[:, 0:1], AF.Identity,
                                 bias=scr[:, 1:2], scale=1.0)

            # PE warm-up so HAM reaches 8/8 right as real matmuls arrive
            pswarm = pspool.tile([128, SBLK], f32, tag="ps", name="pswarm")
            for r in range(11):
                nc.tensor.matmul(pswarm[:, 0:CHUNK], scr[:, 0:128], scr[:],
                                 start=True, stop=True, skip_group_check=True)

            def clip_v(out_ap, x_ap, hi, lo):
                nc.vector.tensor_scalar(out_ap, x_ap, hi, lo, OP.min, OP.max)

            def clip_g(out_ap, x_ap, hi, lo):
                nc.gpsimd.tensor_scalar(out_ap, x_ap, hi, lo, OP.min, OP.max)

            def mm_group(ps, pA, pB, sil_ap, lo, hi, off):
                """Chunks [lo,hi) of ps; feature APs indexed from chunk off."""
                for c in range(lo, hi):
                    f = c - off
                    nc.tensor.matmul(ps[:, c * CHUNK:(c + 1) * CHUNK], dpA,
                                     pA[:, 0:2, f * CHUNK:(f + 1) * CHUNK],
                                     start=True, stop=False, perf_mode=DRm,
                                     skip_group_check=True)
                for c in range(lo, hi):
                    f = c - off
                    nc.tensor.matmul(ps[:, c * CHUNK:(c + 1) * CHUNK], dpB,
                                     pB[:, 0:2, f * CHUNK:(f + 1) * CHUNK],
                                     start=False, stop=False, perf_mode=DRm,
                                     skip_group_check=True)
                for c in range(lo, hi):
                    f = c - off
                    nc.tensor.matmul(ps[:, c * CHUNK:(c + 1) * CHUNK], dsil,
                                     sil_ap[:, f * CHUNK:(f + 1) * CHUNK],
                                     start=False, stop=True, skip_group_check=True)

            def evac(y, ps, a, b, sp):
                nc.scalar.activation(y[:, a:sp], ps[:, a:sp], AF.Identity,
                                     bias=cp[:, 5:6], scale=1.0)
                nc.vector.tensor_scalar(y[:, sp:b], ps[:, sp:b],
                                        cp[:, 5:6], None, OP.add)

            def features(pA, pB, x_ap, cl, g_v4):
                """v1 on GpSimd; v2,v3 on V; v4 on G if g_v4 else V."""
                clip_g(pA[:, 0, cl], x_ap, -0.5, -1.0)
                clip_v(pA[:, 1, cl], x_ap, 0.0, -0.5)
                clip_v(pB[:, 0, cl], x_ap, 0.5, 0.0)
                if g_v4:
                    clip_g(pB[:, 1, cl], x_ap, 1.0, 0.5)
                else:
                    clip_v(pB[:, 1, cl], x_ap, 1.0, 0.5)

            # ---- SB0: two halves for a fast ramp ----
            ps0 = pspool.tile([128, SBLK], f32, tag="ps", name="ps0")
            y0 = ypool.tile([128, SBLK], f16, tag="y", name="y0")
            pA0 = apool.tile([128, 2, SBLK], f8, tag="pA", name="pA0")
            pB0 = bpool.tile([128, 2, SBLK], f8, tag="pB", name="pB0")
            for h in range(2):
                cl = slice(h * 1024, (h + 1) * 1024)
                features(pA0, pB0, x0[:, cl], cl, g_v4=False)
                sl = spool.tile([128, 1024], f16, tag=f"sil0{h}", name=f"sil0{h}")
                nc.scalar.activation(sl[:], x0[:, cl], AF.Silu)
                mm_group(ps0, pA0[:, :, cl], pB0[:, :, cl], sl[:],
                         2 * h, 2 * h + 2, 2 * h)
            evac(y0, ps0, 0, SBLK, SPLIT)
            nc.sync.dma_start(yt[:, 0:SBLK], y0[:])

            # ---- SB1 ----
            ps1 = pspool.tile([128, SBLK], f32, tag="ps", name="ps1")
            y1 = ypool.tile([128, SBLK], f16, tag="y", name="y1")
            pA1 = apool.tile([128, 2, SBLK], f8, tag="pA", name="pA1")
            pB1 = bpool.tile([128, 2, SBLK], f8, tag="pB", name="pB1")
            sil1 = spool.tile([128, SBLK], f16, tag="sil1", name="sil1")
            nc.scalar.activation(sil1[:], x1[:], AF.Silu)
            features(pA1, pB1, x1[:], slice(0, SBLK), g_v4=True)
            mm_group(ps1, pA1, pB1, sil1[:], 0, 4, 0)
            evac(y1, ps1, 0, SBLK, SPLIT)
            nc.sync.dma_start(yt[:, SBLK:2 * SBLK], y1[:])

            # ---- SB2 + SB3 ----
            sil23 = spool.tile([128, 2 * SBLK], f16, tag="sil23", name="sil23")
            nc.scalar.activation(sil23[:], x23[:], AF.Silu)
            pA2 = apool.tile([128, 2, SBLK], f8, tag="pA", name="pA2")
            pB2 = bpool.tile([128, 2, SBLK], f8, tag="pB", name="pB2")
            features(pA2, pB2, x23[:, 0:SBLK], slice(0, SBLK), g_v4=False)
            pA3 = apool.tile([128, 2, SBLK], f8, tag="pA", name="pA3")
            pB3 = bpool.tile([128, 2, SBLK], f8, tag="pB", name="pB3")
            features(pA3, pB3, x23[:, SBLK:2 * SBLK], slice(0, SBLK), g_v4=True)

            ps2 = pspool.tile([128, SBLK], f32, tag="ps", name="ps2")
            y2 = ypool.tile([128, SBLK], f16, tag="y", name="y2")
            mm_group(ps2, pA2, pB2, sil23[:, 0:SBLK], 0, 4, 0)
            evac(y2, ps2, 0, SBLK, SPLIT)
            nc.sync.dma_start(yt[:, 2 * SBLK:3 * SBLK], y2[:])

            ps3 = pspool.tile([128, SBLK], f32, tag="ps", name="ps3")
            y3 = ypool.tile([128, SBLK], f16, tag="y", name="y3")
            for h in range(2):
                cf = slice(h * 1024, (h + 1) * 1024)
                mm_group(ps3, pA3[:, :, cf], pB3[:, :, cf],
                         sil23[:, SBLK + h * 1024:SBLK + (h + 1) * 1024],
                         2 * h, 2 * h + 2, 2 * h)
                evac(y3, ps3, h * 1024, (h + 1) * 1024, h * 1024 + SPLIT_H)
                nc.sync.dma_start(
                    yt[:, 3 * SBLK + h * 1024:3 * SBLK + (h + 1) * 1024],
                    y3[:, h * 1024:(h + 1) * 1024])
    nc.compile()
    return nc


def _host_prep(X, coeffs, W, b):
    """cpack [O,8] fp32 (col5 = compensated bias), dstat16 [O,128] f16 diag(W0),
    dstat8 [O,512] f8 = DoubleRow diag pairs (g1,g2 | g3,g4)."""
    c = coeffs.astype(np.float64)
    W64 = W.astype(np.float64)
    b64 = b.astype(np.float64)
    m = 2.0 * (c[:, 1:] - c[:, :-1])          # [O, 4] segment slopes
    w1 = W64[:, 1]
    gam = w1[:, None] * m                      # [O, 4]
    s = np.array([-1.0, -0.5, 0.0, 0.5])
    A = b64 + w1 * c[:, 0] - (gam * s[None, :]).sum(1)
    # minimax compensation of the fp8e4 weight quantization (RNE):
    # err(x) = sum_j dg_j v_j(x) is piecewise linear with vertices at the
    # knots; evaluate there and recenter via the bias.
    f8np = ml_dtypes.float8_e4m3
    d = gam.astype(f8np).astype(np.float64) - gam   # [O, 4]
    vk = np.array([  # v_j at x = -1, -0.5, 0, 0.5, 1
        [-1.0, -0.5, 0.0, 0.5],
        [-0.5, -0.5, 0.0, 0.5],
        [-0.5, 0.0, 0.0, 0.5],
        [-0.5, 0.0, 0.5, 0.5],
        [-0.5, 0.0, 0.5, 1.0],
    ])
    e = d @ vk.T                                    # [O, 5]
    A = A - (e.max(1) + e.min(1)) / 2

    cpack = np.zeros((OUT_F, 8), dtype=np.float32)
    cpack[:, 0] = W64[:, 0]
    cpack[:, 1:5] = gam
    cpack[:, 5] = A

    eye = np.eye(128)
    dstat16 = np.zeros((OUT_F, 128), dtype=np.float16)
    dstat8 = np.zeros((OUT_F, 512), dtype=f8np)
    for cidx in range(N_CORES):
        sl = slice(cidx * PER, (cidx + 1) * PER)
        dstat16[sl] = (eye * W64[sl, 0][:, None]).astype(np.float16)
        for j in range(4):
            dstat8[sl, j * 128:(j + 1) * 128] = \
                (eye * gam[sl, j][:, None]).astype(f8np)
    return cpack, dstat16, dstat8


def kernel(X, coeffs, W, b):
    global _nc_cache
    if _nc_cache is None:
        _nc_cache = _build()
    nc = _nc_cache

    cpack, dstat16, dstat8 = _host_prep(X, coeffs, W, b)
    X16 = X.astype(np.float16)
    in_maps = []
    for cidx in range(N_CORES):
        sl = slice(cidx * PER, (cidx + 1) * PER)
        in_maps.append({
            "xt": np.ascontiguousarray(X16[:, sl].T),
            "cpack": np.ascontiguousarray(cpack[sl]),
            "dstat16": np.ascontiguousarray(dstat16[sl]),
            "dstat8": np.ascontiguousarray(dstat8[sl]),
        })

    res = run_bass_kernel_spmd(nc, in_maps, core_ids=list(range(N_CORES)))
    Y = np.empty((B, OUT_F), dtype=np.float32)
    for cidx in range(N_CORES):
        sl = slice(cidx * PER, (cidx + 1) * PER)
        Y[:, sl] = res.results[cidx]["yt"].T.astype(np.float32)
    return Y


# revision 11
# speedup vs baseline: 1.2033x; 1.2033x over previous
"""KAN layer (B=8192, IN_F=OUT_F=1024, GRID=5) on 8 Trainium2 cores.

Math: Y[b,o] = W0[o]*silu(x) + spline_o(clip(x,-1,1)) + b[o], x = X[b,o].
The degree-1 B-spline is evaluated in the *segment* basis
    spline(clip(x)) = A''[o] + sum_j gamma_j[o] * v_j(x),
    v_j(x) = clip(x, s_{j-1}, s_j),  knots s = (-1,-0.5,0,0.5,1),
    gamma_j = w1 * m_j (segment slopes),
so each map is a 2-op tensor_scalar clip straight from x.

Sharding: edges across the 8 cores (128 edges/core, full batch 8192 on the
free dim).  Per core TensorE does a per-edge diagonal combine of 5 feature
maps into PSUM per 512-column chunk: v2,v3 ride ONE fp8e4 DoubleRow matmul
(2 maps/pass, diag pair prebuilt on host), v1,v4 and silu are fp16
matmuls.  ScalarE: silu + most of the PSUM evacuation (Identity+bias);
VectorE: the 4 clips + the evac remainder.  I/O fp16; fp8 weight
quantization is minimax-compensated into the per-edge bias on host.
"""
import sys

for _p in ("/root/.axon_site", "/root/.axon_site/_ro/trn_rl_repo", "/root/.axon_site/_ro/pypackages"):
    if _p not in sys.path:
        sys.path.append(_p)

import numpy as np
import ml_dtypes

import concourse.bacc as bacc
import concourse.tile as tile
from concourse import mybir
from concourse.bass_utils import run_bass_kernel_spmd

B, IN_F, OUT_F, GRID = 8192, 1024, 1024, 5
N_CORES = 8
PER = OUT_F // N_CORES          # 128 edges per core
NB = B                          # 8192 batch columns per core
SBLK = 2048                     # superblock columns
NSB = NB // SBLK                # 4 superblocks
CHUNK = 512                     # one PSUM bank of fp32
SPLIT = 1568                    # evac columns on ScalarE per superblock
SPLIT_H = 784                   # evac columns on ScalarE per half-superblock

_nc_cache = None


def _build():
    f32 = mybir.dt.float32
    f16 = mybir.dt.float16
    f8 = mybir.dt.float8e4
    AF = mybir.ActivationFunctionType
    OP = mybir.AluOpType
    DRm = mybir.MatmulPerfMode.DoubleRow

    nc = bacc.Bacc("TRN2", target_bir_lowering=False, debug=False)
    xt = nc.dram_tensor("xt", [PER, NB], f16, kind="ExternalInput").ap()
    cpack = nc.dram_tensor("cpack", [PER, 8], f32, kind="ExternalInput").ap()
    dstat16 = nc.dram_tensor("dstat16", [PER, 384], f16, kind="ExternalInput").ap()
    dstat8 = nc.dram_tensor("dstat8", [PER, 256], f8, kind="ExternalInput").ap()
    yt = nc.dram_tensor("yt", [PER, NB], f16, kind="ExternalOutput").ap()

    with tile.TileContext(nc) as tc:
        with tc.tile_pool(name="const", bufs=1) as cpool, \
             tc.tile_pool(name="xin", bufs=1) as xpool, \
             tc.tile_pool(name="sil", bufs=1) as spool, \
             tc.tile_pool(name="pA", bufs=2) as apool, \
             tc.tile_pool(name="pB", bufs=2) as bpool, \
             tc.tile_pool(name="yout", bufs=2) as ypool, \
             tc.tile_pool(name="ps", bufs=2, space="PSUM") as pspool:
            # consts via the scalar-HWDGE ring; dp (first matmul dep) first
            dp = cpool.tile([128, 2, 128], f8)
            nc.scalar.dma_start(dp[:].rearrange("p a b -> p (a b)"), dstat8[:, :])
            d16 = cpool.tile([128, 384], f16)
            nc.scalar.dma_start(d16[:], dstat16[:, :])
            cp = cpool.tile([128, 8], f32)
            nc.scalar.dma_start(cp[:], cpack[:, :])
            dp23 = dp[:, 0:2, :]
            dsil = d16[:, 0:128]
            dv1 = d16[:, 128:256]
            dv4 = d16[:, 256:384]

            # input loads on the sync/HWDGE ring; small first chunk
            x0 = xpool.tile([128, SBLK], f16, tag="x0", name="x0")
            nc.sync.dma_start(x0[:, 0:1024], xt[:, 0:1024])
            nc.sync.dma_start(x0[:, 1024:SBLK], xt[:, 1024:SBLK])
            x1 = xpool.tile([128, SBLK], f16, tag="x1", name="x1")
            nc.sync.dma_start(x1[:], xt[:, SBLK:2 * SBLK])
            x23 = xpool.tile([128, 2 * SBLK], f16, tag="x23", name="x23")
            nc.sync.dma_start(x23[:], xt[:, 2 * SBLK:4 * SBLK])

            scr = cpool.tile([128, CHUNK], f16)
            nc.vector.memset(scr[:], 0.25)
            # trigger ACT table sets during the DMA ramp
            dum = cpool.tile([128, 2], f16)
            nc.scalar.activation(dum[:, 0:1], scr[:, 0:1], AF.Silu)
            nc.scalar.activation(dum[:, 1:2], scr[:, 0:1], AF.Identity,
                                 bias=scr[:, 1:2], scale=1.0)

            # PE warm-up so HAM reaches 8/8 right as real matmuls arrive
            pswarm = pspool.tile([128, SBLK], f32, tag="ps", name="pswarm")
            for r in range(11):
                nc.tensor.matmul(pswarm[:, 0:CHUNK], scr[:, 0:128], scr[:],
                                 start=True, stop=True, skip_group_check=True)

            def clip_v(out_ap, x_ap, hi, lo):
                nc.vector.tensor_scalar(out_ap, x_ap, hi, lo, OP.min, OP.max)

            def mm_group(ps, p23, v1c, v4c, sil_ap, lo, hi, off):
                """Chunks [lo,hi) of ps; feature APs indexed from chunk off."""
                for c in range(lo, hi):
                    f = c - off
                    nc.tensor.matmul(ps[:, c * CHUNK:(c + 1) * CHUNK], dp23,
                                     p23[:, 0:2, f * CHUNK:(f + 1) * CHUNK],
                                     start=True, stop=False, perf_mode=DRm,
                                     skip_group_check=True)
                for c in range(lo, hi):
                    f = c - off
                    nc.tensor.matmul(ps[:, c * CHUNK:(c + 1) * CHUNK], dv1,
                                     v1c[:, f * CHUNK:(f + 1) * CHUNK],
                                     start=False, stop=False, skip_group_check=True)
                for c in range(lo, hi):
                    f = c - off
                    nc.tensor.matmul(ps[:, c * CHUNK:(c + 1) * CHUNK], dv4,
                                     v4c[:, f * CHUNK:(f + 1) * CHUNK],
                                     start=False, stop=False, skip_group_check=True)
                for c in range(lo, hi):
                    f = c - off
                    nc.tensor.matmul(ps[:, c * CHUNK:(c + 1) * CHUNK], dsil,
                                     sil_ap[:, f * CHUNK:(f + 1) * CHUNK],
                                     start=False, stop=True, skip_group_check=True)

            def evac(y, ps, a, b, sp):
                nc.scalar.activation(y[:, a:sp], ps[:, a:sp], AF.Identity,
                                     bias=cp[:, 5:6], scale=1.0)
                nc.vector.tensor_scalar(y[:, sp:b], ps[:, sp:b],
                                        cp[:, 5:6], None, OP.add)

            def features(p23, v1t, v4t, x_ap, cl):
                clip_v(p23[:, 0, cl], x_ap, 0.0, -0.5)
                clip_v(p23[:, 1, cl], x_ap, 0.5, 0.0)
                clip_v(v1t[:, cl], x_ap, -0.5, -1.0)
                clip_v(v4t[:, cl], x_ap, 1.0, 0.5)

            # ---- SB0: two halves for a fast ramp ----
            ps0 = pspool.tile([128, SBLK], f32, tag="ps", name="ps0")
            y0 = ypool.tile([128, SBLK], f16, tag="y", name="y0")
            p23_0 = apool.tile([128, 2, SBLK], f8, tag="pA", name="p23_0")
            v1_0 = bpool.tile([128, SBLK], f16, tag="v1", name="v1_0")
            v4_0 = bpool.tile([128, SBLK], f16, tag="v4", name="v4_0")
            for h in range(2):
                cl = slice(h * 1024, (h + 1) * 1024)
                features(p23_0, v1_0, v4_0, x0[:, cl], cl)
                sl = spool.tile([128, 1024], f16, tag=f"sil0{h}", name=f"sil0{h}")
                nc.scalar.activation(sl[:], x0[:, cl], AF.Silu)
                mm_group(ps0, p23_0[:, :, cl], v1_0[:, cl], v4_0[:, cl], sl[:],
                         2 * h, 2 * h + 2, 2 * h)
            evac(y0, ps0, 0, SBLK, SPLIT)
            nc.sync.dma_start(yt[:, 0:SBLK], y0[:])

            # ---- SB1 ----
            ps1 = pspool.tile([128, SBLK], f32, tag="ps", name="ps1")
            y1 = ypool.tile([128, SBLK], f16, tag="y", name="y1")
            p23_1 = apool.tile([128, 2, SBLK], f8, tag="pA", name="p23_1")
            v1_1 = bpool.tile([128, SBLK], f16, tag="v1", name="v1_1")
            v4_1 = bpool.tile([128, SBLK], f16, tag="v4", name="v4_1")
            sil1 = spool.tile([128, SBLK], f16, tag="sil1", name="sil1")
            nc.scalar.activation(sil1[:], x1[:], AF.Silu)
            features(p23_1, v1_1, v4_1, x1[:], slice(0, SBLK))
            mm_group(ps1, p23_1, v1_1, v4_1, sil1[:], 0, 4, 0)
            evac(y1, ps1, 0, SBLK, SPLIT)
            nc.sync.dma_start(yt[:, SBLK:2 * SBLK], y1[:])

            # ---- SB2 + SB3 ----
            sil23 = spool.tile([128, 2 * SBLK], f16, tag="sil23", name="sil23")
            nc.scalar.activation(sil23[:], x23[:], AF.Silu)
            p23_2 = apool.tile([128, 2, SBLK], f8, tag="pA", name="p23_2")
            v1_2 = bpool.tile([128, SBLK], f16, tag="v1", name="v1_2")
            v4_2 = bpool.tile([128, SBLK], f16, tag="v4", name="v4_2")
            features(p23_2, v1_2, v4_2, x23[:, 0:SBLK], slice(0, SBLK))
            p23_3 = apool.tile([128, 2, SBLK], f8, tag="pA", name="p23_3")
            v1_3 = bpool.tile([128, SBLK], f16, tag="v1", name="v1_3")
            v4_3 = bpool.tile([128, SBLK], f16, tag="v4", name="v4_3")
            features(p23_3, v1_3, v4_3, x23[:, SBLK:2 * SBLK], slice(0, SBLK))

            ps2 = pspool.tile([128, SBLK], f32, tag="ps", name="ps2")
            y2 = ypool.tile([128, SBLK], f16, tag="y", name="y2")
            mm_group(ps2, p23_2, v1_2, v4_2, sil23[:, 0:SBLK], 0, 4, 0)
            evac(y2, ps2, 0, SBLK, SPLIT)
            nc.sync.dma_start(yt[:, 2 * SBLK:3 * SBLK], y2[:])

            ps3 = pspool.tile([128, SBLK], f32, tag="ps", name="ps3")
            y3 = ypool.tile([128, SBLK], f16, tag="y", name="y3")
            for h in range(2):
                cf = slice(h * 1024, (h + 1) * 1024)
                mm_group(ps3, p23_3[:, :, cf], v1_3[:, cf], v4_3[:, cf],
                         sil23[:, SBLK + h * 1024:SBLK + (h + 1) * 1024],
                         2 * h, 2 * h + 2, 2 * h)
                evac(y3, ps3, h * 1024, (h + 1) * 1024, h * 1024 + SPLIT_H)
                nc.sync.dma_start(
                    yt[:, 3 * SBLK + h * 1024:3 * SBLK + (h + 1) * 1024],
                    y3[:, h * 1024:(h + 1) * 1024])
    nc.compile()
    return nc


def _host_prep(X, coeffs, W, b):
    """cpack [O,8] fp32 (col5 = compensated bias), dstat16 [O,128] f16 diag(W0),
    dstat8 [O,512] f8 = DoubleRow diag pairs (g1,g2 | g3,g4)."""
    c = coeffs.astype(np.float64)
    W64 = W.astype(np.float64)
    b64 = b.astype(np.float64)
    m = 2.0 * (c[:, 1:] - c[:, :-1])          # [O, 4] segment slopes
    w1 = W64[:, 1]
    gam = w1[:, None] * m                      # [O, 4]
    s = np.array([-1.0, -0.5, 0.0, 0.5])
    A = b64 + w1 * c[:, 0] - (gam * s[None, :]).sum(1)
    # minimax compensation of the fp8e4 weight quantization of g2,g3 (RNE)
    f8np = ml_dtypes.float8_e4m3
    d2 = gam[:, 1].astype(f8np).astype(np.float64) - gam[:, 1]
    d3 = gam[:, 2].astype(f8np).astype(np.float64) - gam[:, 2]
    e = np.stack([-0.5 * d2, np.zeros_like(d2), 0.5 * d3], 1)
    A = A - (e.max(1) + e.min(1)) / 2

    cpack = np.zeros((OUT_F, 8), dtype=np.float32)
    cpack[:, 0] = W64[:, 0]
    cpack[:, 1:5] = gam
    cpack[:, 5] = A

    eye = np.eye(128)
    dstat16 = np.zeros((OUT_F, 384), dtype=np.float16)
    dstat8 = np.zeros((OUT_F, 256), dtype=f8np)
    for cidx in range(N_CORES):
        sl = slice(cidx * PER, (cidx + 1) * PER)
        dstat16[sl, 0:128] = (eye * W64[sl, 0][:, None]).astype(np.float16)
        dstat16[sl, 128:256] = (eye * gam[sl, 0][:, None]).astype(np.float16)
        dstat16[sl, 256:384] = (eye * gam[sl, 3][:, None]).astype(np.float16)
        dstat8[sl, 0:128] = (eye * gam[sl, 1][:, None]).astype(f8np)
        dstat8[sl, 128:256] = (eye * gam[sl, 2][:, None]).astype(f8np)
    return cpack, dstat16, dstat8


def kernel(X, coeffs, W, b):
    global _nc_cache
    if _nc_cache is None:
        _nc_cache = _build()
    nc = _nc_cache

    cpack, dstat16, dstat8 = _host_prep(X, coeffs, W, b)
    X16 = X.astype(np.float16)
    in_maps = []
    for cidx in range(N_CORES):
        sl = slice(cidx * PER, (cidx + 1) * PER)
        in_maps.append({
            "xt": np.ascontiguousarray(X16[:, sl].T),
            "cpack": np.ascontiguousarray(cpack[sl]),
            "dstat16": np.ascontiguousarray(dstat16[sl]),
            "dstat8": np.ascontiguousarray(dstat8[sl]),
        })

    res = run_bass_kernel_spmd(nc, in_maps, core_ids=list(range(N_CORES)))
    Y = np.empty((B, OUT_F), dtype=np.float32)
    for cidx in range(N_CORES):
        sl = slice(cidx * PER, (cidx + 1) * PER)
        Y[:, sl] = res.results[cidx]["yt"].T.astype(np.float32)
    return Y


# revision 12
# speedup vs baseline: 1.2169x; 1.0113x over previous
"""KAN layer (B=8192, IN_F=OUT_F=1024, GRID=5) on 8 Trainium2 cores.

Math: Y[b,o] = W0[o]*silu(x) + spline_o(clip(x,-1,1)) + b[o], x = X[b,o].
The degree-1 B-spline is evaluated in the *segment* basis
    spline(clip(x)) = A''[o] + sum_j gamma_j[o] * v_j(x),
    v_j(x) = clip(x, s_{j-1}, s_j),  knots s = (-1,-0.5,0,0.5,1),
    gamma_j = w1 * m_j (segment slopes),
so each map is a 2-op tensor_scalar clip straight from x.

Sharding: edges across the 8 cores (128 edges/core, full batch 8192 on the
free dim).  Per core TensorE does a per-edge diagonal combine of 5 feature
maps into PSUM per 512-column chunk: v2,v3 ride ONE fp8e4 DoubleRow matmul
(2 maps/pass, diag pair prebuilt on host), v1,v4 and silu are fp16
matmuls.  ScalarE: silu + most of the PSUM evacuation (Identity+bias);
VectorE: the 4 clips + the evac remainder.  I/O fp16; fp8 weight
quantization is minimax-compensated into the per-edge bias on host.
"""
import sys

for _p in ("/root/.axon_site", "/root/.axon_site/_ro/trn_rl_repo", "/root/.axon_site/_ro/pypackages"):
    if _p not in sys.path:
        sys.path.append(_p)

import numpy as np
import ml_dtypes

import concourse.bacc as bacc
import concourse.tile as tile
from concourse import mybir
from concourse.bass_utils import run_bass_kernel_spmd

B, IN_F, OUT_F, GRID = 8192, 1024, 1024, 5
N_CORES = 8
PER = OUT_F // N_CORES          # 128 edges per core
NB = B                          # 8192 batch columns per core
SBLK = 2048                     # superblock columns
NSB = NB // SBLK                # 4 superblocks
CHUNK = 512                     # one PSUM bank of fp32
SPLIT = 1792                    # evac columns on ScalarE (SB1/SB2)
SPLIT_H = 1024                  # SB0/SB3 evacs run fully on ScalarE

_nc_cache = None


def _build():
    f32 = mybir.dt.float32
    f16 = mybir.dt.float16
    f8 = mybir.dt.float8e4
    AF = mybir.ActivationFunctionType
    OP = mybir.AluOpType
    DRm = mybir.MatmulPerfMode.DoubleRow

    nc = bacc.Bacc("TRN2", target_bir_lowering=False, debug=False)
    xt = nc.dram_tensor("xt", [PER, NB], f16, kind="ExternalInput").ap()
    cpack = nc.dram_tensor("cpack", [PER, 8], f32, kind="ExternalInput").ap()
    dstat16 = nc.dram_tensor("dstat16", [PER, 384], f16, kind="ExternalInput").ap()
    dstat8 = nc.dram_tensor("dstat8", [PER, 256], f8, kind="ExternalInput").ap()
    yt = nc.dram_tensor("yt", [PER, NB], f16, kind="ExternalOutput").ap()

    with tile.TileContext(nc) as tc:
        with tc.tile_pool(name="const", bufs=1) as cpool, \
             tc.tile_pool(name="xin", bufs=1) as xpool, \
             tc.tile_pool(name="sil", bufs=1) as spool, \
             tc.tile_pool(name="pA", bufs=2) as apool, \
             tc.tile_pool(name="pB", bufs=2) as bpool, \
             tc.tile_pool(name="yout", bufs=2) as ypool, \
             tc.tile_pool(name="ps", bufs=2, space="PSUM") as pspool:
            # consts via the scalar-HWDGE ring; dp (first matmul dep) first
            dp = cpool.tile([128, 2, 128], f8)
            nc.scalar.dma_start(dp[:].rearrange("p a b -> p (a b)"), dstat8[:, :])
            d16 = cpool.tile([128, 384], f16)
            nc.scalar.dma_start(d16[:], dstat16[:, :])
            cp = cpool.tile([128, 8], f32)
            nc.scalar.dma_start(cp[:], cpack[:, :])
            dp23 = dp[:, 0:2, :]
            dsil = d16[:, 0:128]
            dv1 = d16[:, 128:256]
            dv4 = d16[:, 256:384]

            # input loads on the sync/HWDGE ring; small first chunk
            x0 = xpool.tile([128, SBLK], f16, tag="x0", name="x0")
            nc.sync.dma_start(x0[:, 0:1024], xt[:, 0:1024])
            nc.sync.dma_start(x0[:, 1024:SBLK], xt[:, 1024:SBLK])
            x1 = xpool.tile([128, SBLK], f16, tag="x1", name="x1")
            nc.sync.dma_start(x1[:], xt[:, SBLK:2 * SBLK])
            x23 = xpool.tile([128, 2 * SBLK], f16, tag="x23", name="x23")
            nc.sync.dma_start(x23[:], xt[:, 2 * SBLK:4 * SBLK])

            scr = cpool.tile([128, CHUNK], f16)
            nc.vector.memset(scr[:], 0.25)
            # trigger ACT table sets during the DMA ramp
            dum = cpool.tile([128, 2], f16)
            nc.scalar.activation(dum[:, 0:1], scr[:, 0:1], AF.Silu)
            nc.scalar.activation(dum[:, 1:2], scr[:, 0:1], AF.Identity,
                                 bias=scr[:, 1:2], scale=1.0)

            # PE warm-up so HAM reaches 8/8 right as real matmuls arrive
            pswarm = pspool.tile([128, SBLK], f32, tag="ps", name="pswarm")
            for r in range(11):
                nc.tensor.matmul(pswarm[:, 0:CHUNK], scr[:, 0:128], scr[:],
                                 start=True, stop=True, skip_group_check=True)

            def clip_v(out_ap, x_ap, hi, lo):
                nc.vector.tensor_scalar(out_ap, x_ap, hi, lo, OP.min, OP.max)

            def mm_group(ps, p23, v1c, v4c, sil_ap, lo, hi, off):
                """Chunks [lo,hi) of ps; feature APs indexed from chunk off."""
                for c in range(lo, hi):
                    f = c - off
                    nc.tensor.matmul(ps[:, c * CHUNK:(c + 1) * CHUNK], dp23,
                                     p23[:, 0:2, f * CHUNK:(f + 1) * CHUNK],
                                     start=True, stop=False, perf_mode=DRm,
                                     skip_group_check=True)
                for c in range(lo, hi):
                    f = c - off
                    nc.tensor.matmul(ps[:, c * CHUNK:(c + 1) * CHUNK], dv1,
                                     v1c[:, f * CHUNK:(f + 1) * CHUNK],
                                     start=False, stop=False, skip_group_check=True)
                for c in range(lo, hi):
                    f = c - off
                    nc.tensor.matmul(ps[:, c * CHUNK:(c + 1) * CHUNK], dv4,
                                     v4c[:, f * CHUNK:(f + 1) * CHUNK],
                                     start=False, stop=False, skip_group_check=True)
                for c in range(lo, hi):
                    f = c - off
                    nc.tensor.matmul(ps[:, c * CHUNK:(c + 1) * CHUNK], dsil,
                                     sil_ap[:, f * CHUNK:(f + 1) * CHUNK],
                                     start=False, stop=True, skip_group_check=True)

            def evac(y, ps, a, b, sp):
                nc.scalar.activation(y[:, a:sp], ps[:, a:sp], AF.Identity,
                                     bias=cp[:, 5:6], scale=1.0)
                if sp < b:
                    nc.vector.tensor_scalar(y[:, sp:b], ps[:, sp:b],
                                            cp[:, 5:6], None, OP.add)

            def features(p23, v1t, v4t, x_ap, cl):
                clip_v(p23[:, 0, cl], x_ap, 0.0, -0.5)
                clip_v(p23[:, 1, cl], x_ap, 0.5, 0.0)
                clip_v(v1t[:, cl], x_ap, -0.5, -1.0)
                clip_v(v4t[:, cl], x_ap, 1.0, 0.5)

            # ---- SB0: two halves for a fast ramp ----
            ps0 = pspool.tile([128, SBLK], f32, tag="ps", name="ps0")
            y0 = ypool.tile([128, SBLK], f16, tag="y", name="y0")
            p23_0 = apool.tile([128, 2, SBLK], f8, tag="pA", name="p23_0")
            v1_0 = bpool.tile([128, SBLK], f16, tag="v1", name="v1_0")
            v4_0 = bpool.tile([128, SBLK], f16, tag="v4", name="v4_0")
            for h in range(2):
                cl = slice(h * 1024, (h + 1) * 1024)
                features(p23_0, v1_0, v4_0, x0[:, cl], cl)
                sl = spool.tile([128, 1024], f16, tag=f"sil0{h}", name=f"sil0{h}")
                nc.scalar.activation(sl[:], x0[:, cl], AF.Silu)
                mm_group(ps0, p23_0[:, :, cl], v1_0[:, cl], v4_0[:, cl], sl[:],
                         2 * h, 2 * h + 2, 2 * h)
            evac(y0, ps0, 0, SBLK, SBLK)
            nc.sync.dma_start(yt[:, 0:SBLK], y0[:])

            # ---- SB1 ----
            ps1 = pspool.tile([128, SBLK], f32, tag="ps", name="ps1")
            y1 = ypool.tile([128, SBLK], f16, tag="y", name="y1")
            p23_1 = apool.tile([128, 2, SBLK], f8, tag="pA", name="p23_1")
            v1_1 = bpool.tile([128, SBLK], f16, tag="v1", name="v1_1")
            v4_1 = bpool.tile([128, SBLK], f16, tag="v4", name="v4_1")
            sil1 = spool.tile([128, SBLK], f16, tag="sil1", name="sil1")
            nc.scalar.activation(sil1[:], x1[:], AF.Silu)
            features(p23_1, v1_1, v4_1, x1[:], slice(0, SBLK))
            mm_group(ps1, p23_1, v1_1, v4_1, sil1[:], 0, 4, 0)
            evac(y1, ps1, 0, SBLK, SPLIT)
            nc.sync.dma_start(yt[:, SBLK:2 * SBLK], y1[:])

            # ---- SB2 + SB3 ----
            sil23 = spool.tile([128, 2 * SBLK], f16, tag="sil23", name="sil23")
            nc.scalar.activation(sil23[:], x23[:], AF.Silu)
            p23_2 = apool.tile([128, 2, SBLK], f8, tag="pA", name="p23_2")
            v1_2 = bpool.tile([128, SBLK], f16, tag="v1", name="v1_2")
            v4_2 = bpool.tile([128, SBLK], f16, tag="v4", name="v4_2")
            features(p23_2, v1_2, v4_2, x23[:, 0:SBLK], slice(0, SBLK))
            p23_3 = apool.tile([128, 2, SBLK], f8, tag="pA", name="p23_3")
            v1_3 = bpool.tile([128, SBLK], f16, tag="v1", name="v1_3")
            v4_3 = bpool.tile([128, SBLK], f16, tag="v4", name="v4_3")
            features(p23_3, v1_3, v4_3, x23[:, SBLK:2 * SBLK], slice(0, SBLK))

            ps2 = pspool.tile([128, SBLK], f32, tag="ps", name="ps2")
            y2 = ypool.tile([128, SBLK], f16, tag="y", name="y2")
            mm_group(ps2, p23_2, v1_2, v4_2, sil23[:, 0:SBLK], 0, 4, 0)
            evac(y2, ps2, 0, SBLK, SPLIT)
            nc.sync.dma_start(yt[:, 2 * SBLK:3 * SBLK], y2[:])

            ps3 = pspool.tile([128, SBLK], f32, tag="ps", name="ps3")
            y3 = ypool.tile([128, SBLK], f16, tag="y", name="y3")
            for h in range(2):
                cf = slice(h * 1024, (h + 1) * 1024)
                mm_group(ps3, p23_3[:, :, cf], v1_3[:, cf], v4_3[:, cf],
                         sil23[:, SBLK + h * 1024:SBLK + (h + 1) * 1024],
                         2 * h, 2 * h + 2, 2 * h)
                evac(y3, ps3, h * 1024, (h + 1) * 1024, (h + 1) * 1024)
                if h == 0:
                    nc.sync.dma_start(yt[:, 3 * SBLK:3 * SBLK + 1024],
                                      y3[:, 0:1024])
                else:
                    nc.sync.dma_start(yt[:, 3 * SBLK + 1024:3 * SBLK + 1536],
                                      y3[:, 1024:1536])
                    nc.sync.dma_start(yt[:, 3 * SBLK + 1536:4 * SBLK],
                                      y3[:, 1536:SBLK])
    nc.compile()
    return nc


def _host_prep(X, coeffs, W, b):
    """cpack [O,8] fp32 (col5 = compensated bias), dstat16 [O,128] f16 diag(W0),
    dstat8 [O,512] f8 = DoubleRow diag pairs (g1,g2 | g3,g4)."""
    c = coeffs.astype(np.float64)
    W64 = W.astype(np.float64)
    b64 = b.astype(np.float64)
    m = 2.0 * (c[:, 1:] - c[:, :-1])          # [O, 4] segment slopes
    w1 = W64[:, 1]
    gam = w1[:, None] * m                      # [O, 4]
    s = np.array([-1.0, -0.5, 0.0, 0.5])
    A = b64 + w1 * c[:, 0] - (gam * s[None, :]).sum(1)
    # minimax compensation of the fp8e4 weight quantization of g2,g3 (RNE)
    f8np = ml_dtypes.float8_e4m3
    d2 = gam[:, 1].astype(f8np).astype(np.float64) - gam[:, 1]
    d3 = gam[:, 2].astype(f8np).astype(np.float64) - gam[:, 2]
    e = np.stack([-0.5 * d2, np.zeros_like(d2), 0.5 * d3], 1)
    A = A - (e.max(1) + e.min(1)) / 2

    cpack = np.zeros((OUT_F, 8), dtype=np.float32)
    cpack[:, 0] = W64[:, 0]
    cpack[:, 1:5] = gam
    cpack[:, 5] = A

    eye = np.eye(128)
    dstat16 = np.zeros((OUT_F, 384), dtype=np.float16)
    dstat8 = np.zeros((OUT_F, 256), dtype=f8np)
    for cidx in range(N_CORES):
        sl = slice(cidx * PER, (cidx + 1) * PER)
        dstat16[sl, 0:128] = (eye * W64[sl, 0][:, None]).astype(np.float16)
        dstat16[sl, 128:256] = (eye * gam[sl, 0][:, None]).astype(np.float16)
        dstat16[sl, 256:384] = (eye * gam[sl, 3][:, None]).astype(np.float16)
        dstat8[sl, 0:128] = (eye * gam[sl, 1][:, None]).astype(f8np)
        dstat8[sl, 128:256] = (eye * gam[sl, 2][:, None]).astype(f8np)
    return cpack, dstat16, dstat8


def kernel(X, coeffs, W, b):
    global _nc_cache
    if _nc_cache is None:
        _nc_cache = _build()
    nc = _nc_cache

    cpack, dstat16, dstat8 = _host_prep(X, coeffs, W, b)
    X16 = X.astype(np.float16)
    in_maps = []
    for cidx in range(N_CORES):
        sl = slice(cidx * PER, (cidx + 1) * PER)
        in_maps.append({
            "xt": np.ascontiguousarray(X16[:, sl].T),
            "cpack": np.ascontiguousarray(cpack[sl]),
            "dstat16": np.ascontiguousarray(dstat16[sl]),
            "dstat8": np.ascontiguousarray(dstat8[sl]),
        })

    res = run_bass_kernel_spmd(nc, in_maps, core_ids=list(range(N_CORES)))
    Y = np.empty((B, OUT_F), dtype=np.float32)
    for cidx in range(N_CORES):
        sl = slice(cidx * PER, (cidx + 1) * PER)
        Y[:, sl] = res.results[cidx]["yt"].T.astype(np.float32)
    return Y
